# revision 1
# baseline (speedup 1.0000x reference)
"""Multi-head attention (B=2, T=2048, D=1024, H=16) on 8 TRN2 NeuronCores.

Sharding: tensor-parallel over heads — core c owns heads (2c, 2c+1).
Each core computes its heads' QKV projection (column-sharded), full attention
for those heads, and a row-sharded O-projection partial; the host sums the 8
bf16 partials in fp32 and adds b_o (with W_o @ b_v folded in, since softmax
rows sum to 1).

Host-side prep (bf16 activations/weights, fp32 biases):
  - x is shipped as xT [D, B*T] so D (the contraction dim) lands on
    partitions.
  - W_qkv head-slices are shipped as lhsT [D, 384] with the softmax scale
    folded into the q columns; W_o slice shipped as rhs [128, D].

On-device layout (per batch):
  qkv_T [128, 2, 2048]: q rows (h0 dims 0-63, h1 dims 64-127) and k rows.
  v is projected directly transposed (per 128-token tile the x slice is the
  stationary operand) into v_sb [128 keys, kt, 2*65] with a ones column per
  head, so the AV matmul's row 64 accumulates the softmax denominators.
  Scores are computed transposed [keys, queries] so softmax exp needs no
  transposes; both heads' scores share one [128, 2, 512] PSUM tile so a
  single 1024-wide exp serves a step; no max subtraction (scores ~ N(0,
  0.33) for this init); normalization broadcasts 1/sum across partitions via
  gpsimd, phase-interleaved across heads for the in-order DVE queue.

Schedule: one flat software pipeline over the 128 (batch, query-block,
key-tile) attention steps (query blocks of 512). Per beat the PE runs
scores[j+2], filler work (QKV/V-projection chunks and O-projection tiles —
in their own PSUM tag, so they never recycle a scores bank), then AV one
beat behind; the scalar engine's exp[j] therefore always has two beats of
slack and the scores PSUM a two-beat reuse distance. AV emission skips the
first beat of each block seam (catching up with a paired emission at the
block end) so the previous block's normalization (single-buffered av PSUM)
can drain. PSUM budget:
scores 2x2 banks, av 2x1, projections 2x1. The epilogue's O tiles copy out
on DVE and ACT in parallel and borrow the drained scores PSUM.
"""

import numpy as np

import concourse.bacc as bacc
import concourse.mybir as mybir
import concourse.tile as tile
from concourse import bass_utils

F32 = mybir.dt.float32
BF16 = mybir.dt.bfloat16

B, T, D, H, DH = 2, 2048, 1024, 16, 64
P = 128
NCORES = 8
HPC = H // NCORES          # heads per core = 2
KT = T // P                # key tiles per batch = 16
QB = 512                   # query block
NQB = T // QB              # query blocks per batch = 4
KD = D // P                # contraction tiles for projections = 8
NBLK = B * NQB             # attention blocks = 8
NJ = NBLK * KT             # global spine steps = 128

MM_DT = BF16               # projection matmul dtype
PV_DT = BF16               # probs + v + q/k dtype


def build_program():
    nc = bacc.Bacc(
        "TRN2",
        target_bir_lowering=False,
        debug=False,
        enable_asserts=False,
        num_devices=NCORES,
    )
    xT = nc.dram_tensor("xT", [D, B * T], MM_DT, kind="ExternalInput").ap()
    wqkvT = nc.dram_tensor("wqkvT", [D, 3 * P], MM_DT, kind="ExternalInput").ap()
    bqk = nc.dram_tensor("bqk", [P, 2], F32, kind="ExternalInput").ap()
    wo = nc.dram_tensor("wo", [P, D], MM_DT, kind="ExternalInput").ap()
    out = nc.dram_tensor("out", [B * T, D], BF16, kind="ExternalOutput").ap()

    with tile.TileContext(nc) as tc:
        _body(tc, xT, wqkvT, bqk, wo, out)
    nc.compile()
    return nc


def _body(tc, xT, wqkvT, bqk, wo, out):
    nc = tc.nc
    ctxs = []

    def pool(name, bufs, space="SBUF"):
        cm = tc.tile_pool(name=name, bufs=bufs, space=space)
        p = cm.__enter__()
        ctxs.append(cm)
        return p

    const = pool("const", 1)
    xp = pool("xp", 6)             # x [128,8,512] bf16 chunk tiles (6 live)
    qkvp = pool("qkvp", 2)
    vp = pool("vp", 2)
    probsp = pool("probsp", 8)     # per-(step, head) probs tiles
    ocatp = pool("ocatp", 2)
    outp = pool("outp", 6)
    recipp = pool("recipp", 4)
    bcp = pool("bcp", 4)
    ps = pool("ps", 1, space="PSUM")   # sc: 4 banks, av: 2, pj: 2

    def ps_sc(name):
        # Both heads' scores for one step side by side: one exp instruction
        # covers 1024 elements, halving ACT instruction count and gates.
        return ps.tile([P, HPC, QB], F32, tag="sc", name=name, bufs=2)

    def ps_pj(name):
        return ps.tile([P, QB], F32, tag="pj", name=name, bufs=2)

    # ---- constants (batched DMAs: one HWDGE hold each) ----
    w_sb = const.tile([P, KD, 3 * P], MM_DT, name="w_sb")
    wqkv_p = wqkvT.rearrange("(ko p) m -> p ko m", p=P)
    nc.sync.dma_start(w_sb[:, 0:2, :], wqkv_p[:, 0:2, :])
    nc.sync.dma_start(w_sb[:, 2:4, :], wqkv_p[:, 2:4, :])
    bqk_sb = const.tile([P, 2], F32, name="bqk_sb")
    wo_sb = const.tile([P, D], MM_DT, name="wo_sb")

    xT_p = xT.rearrange("(ko p) t -> p ko t", p=P)

    # ---------------- per-batch state + work units ----------------
    st = {}
    xc = {}

    def batch_state(b):
        qkvT = qkvp.tile([P, 3, T], PV_DT, tag="qkv", name=f"qkv_{b}")
        v_sb = vp.tile([P, KT, 2 * (DH + 1)], PV_DT, tag="v", name=f"v_{b}")
        v4 = v_sb.rearrange("p t (g c) -> p t g c", g=2)
        nc.vector.memset(v4[:, :, :, DH:DH + 1], 1.0)
        ocat = ocatp.tile([P, T], MM_DT, tag="ocat", name=f"ocat_{b}")
        st[b] = dict(qkvT=qkvT, v4=v4, ocat=ocat)

    def dma_x(b, n):
        """Fetch 512-token chunk n of batch b as two 4-k-tile DMAs."""
        x_t = xp.tile([P, KD, 512], MM_DT, tag="x", name=f"x_{b}_{n}")
        cols = slice(b * T + n * 512, b * T + (n + 1) * 512)
        nc.sync.dma_start(x_t[:, 0:4, :], xT_p[:, 0:4, cols])
        nc.sync.dma_start(x_t[:, 4:KD, :], xT_p[:, 4:KD, cols])
        xc[b, n] = x_t

    def _proj_consume(b, n, m, pq):
        dst = st[b]["qkvT"][:, m, n * 512:(n + 1) * 512]
        if m < 2:
            nc.vector.tensor_scalar_add(dst, pq, bqk_sb[:, m:m + 1])
        else:
            nc.vector.tensor_copy(out=dst, in_=pq)

    def proj(b, n, m):
        """P unit: project chunk n into qkvT[:, m] (8 accum MMs + consumer)."""
        pq = ps_pj(f"qkvps_{b}_{m}_{n}")
        for k in range(KD):
            nc.tensor.matmul(
                pq,
                w_sb[:, k, m * P:(m + 1) * P],
                xc[b, n][:, k, :],
                start=(k == 0),
                stop=(k == KD - 1),
            )
        _proj_consume(b, n, m, pq)

    def proj_qk(b, n):
        """Fused q+k projection of chunk n: both consume x k-tiles as they
        land, so the prologue is paced by one DMA stream, not two passes."""
        pq = [ps_pj(f"qkvps_{b}_0_{n}"),
              ps_sc(f"qkvps_{b}_1_{n}")[:, 0, :]]
        for k in range(KD):
            for m in range(2):
                nc.tensor.matmul(
                    pq[m],
                    w_sb[:, k, m * P:(m + 1) * P],
                    xc[b, n][:, k, :],
                    start=(k == 0),
                    stop=(k == KD - 1),
                )
        for m in range(2):
            _proj_consume(b, n, m, pq[m])

    def vproj(b, n):
        """V unit: project chunk n directly transposed — per 128-token tile,
        the x slice is the stationary operand, so the PSUM comes out
        [tokens, vdims] and no PE transpose is needed."""
        v4 = st[b]["v4"]
        pv = ps_pj(f"vp_{b}_{n}")
        pvt = pv.rearrange("p (t c) -> p t c", t=4)
        for tt in range(4):
            for k in range(KD):
                nc.tensor.matmul(
                    pvt[:, tt, :],
                    xc[b, n][:, k, tt * P:(tt + 1) * P],
                    w_sb[:, k, 2 * P:3 * P],
                    start=(k == 0),
                    stop=(k == KD - 1),
                )
        nc.vector.tensor_copy(
            out=v4[:, 4 * n:4 * n + 4, :, 0:DH],
            in_=pv.bitcast(F32).rearrange("p (t g c) -> p t g c", t=4, g=2),
        )

    def oproj(b, tt, on_act=False, sc_psum=False, dual=False):
        """O unit: project ocat token-tile tt, copy out halves, DMA.

        sc_psum borrows a scores-tag PSUM tile (both halves side by side);
        dual puts one half's copy on DVE and the other on ACT — both only
        safe in the epilogue once the scores/exp streams have drained.
        """
        ocat = st[b]["ocat"]
        po2 = ps_sc(f"op2_{b}_{tt}") if sc_psum else None
        for nn in range(D // 512):
            po = po2[:, nn, :] if sc_psum else ps_pj(f"op_{b}_{tt}_{nn}")
            nc.tensor.matmul(
                po,
                ocat[:, tt * P:(tt + 1) * P],
                wo_sb[:, nn * 512:(nn + 1) * 512],
                start=True,
                stop=True,
            )
            ob = outp.tile([P, 512], BF16, tag="ob", name=f"ob_{b}_{tt}_{nn}")
            if on_act or (dual and nn == 1):
                nc.scalar.activation(ob, po, mybir.ActivationFunctionType.Copy)
            else:
                nc.vector.tensor_copy(out=ob, in_=po)
            nc.sync.dma_start(
                out[b * T + tt * P: b * T + (tt + 1) * P,
                    nn * 512:(nn + 1) * 512], ob)

    # ---------------- attention spine (global steps j = 0..127) -------------
    # step j -> block bi = j // KT -> (b, qb) = divmod(bi, NQB), kt = j % KT
    blocks = [None] * NBLK
    probs = {}   # j -> [tile_h0, tile_h1]

    def block_begin(bi):
        blocks[bi] = [
            ps.tile([DH + 1, QB], F32, tag=f"av{h}",
                    name=f"av_{bi}_{h}", bufs=1) for h in range(HPC)]

    def emit_scores(j):
        bi, kt = divmod(j, KT)
        if blocks[bi] is None:
            block_begin(bi)
        b, qb = divmod(bi, NQB)
        qkvT = st[b]["qkvT"]
        q0 = qb * QB
        s = ps_sc(f"s_{bi}_{kt}")
        for h in range(HPC):
            hs = h * DH
            nc.tensor.matmul(
                s[:, h, :],
                qkvT[hs:hs + DH, 1, kt * P:(kt + 1) * P],
                qkvT[hs:hs + DH, 0, q0:q0 + QB],
                start=True,
                stop=True,
                tile_position=(hs, 0),
            )
        return s

    def emit_exp(j, ps_cur):
        pt = probsp.tile([P, HPC, QB], PV_DT, tag="probs",
                         name=f"pb_{j}", bufs=5)
        nc.scalar.activation(pt, ps_cur,
                             mybir.ActivationFunctionType.Exp)
        probs[j] = pt

    def emit_av(j):
        bi, kt = divmod(j, KT)
        b = bi // NQB
        v4 = st[b]["v4"]
        av = blocks[bi]
        for h in range(HPC):
            nc.tensor.matmul(
                av[h],
                v4[:, kt, h, :],  # [128, 65]
                probs[j][:, h, :],
                start=(kt == 0),
                stop=(kt == KT - 1),
            )
        del probs[j]

    def norm(bi):
        """Normalize both heads of block bi into ocat. Phases interleave so
        the second head's reciprocal is not stuck behind the first head's
        multiply in the in-order DVE queue."""
        b, qb = divmod(bi, NQB)
        ocat = st[b]["ocat"]
        av = blocks[bi]
        recips, bcs = [], []
        for h in range(HPC):
            r = recipp.tile([1, QB], F32, tag="recip", name=f"rc_{bi}_{h}")
            nc.vector.reciprocal(r, av[h][DH:DH + 1, :])
            recips.append(r)
        for h in range(HPC):
            bc = bcp.tile([DH, QB], F32, tag="bc", name=f"bc_{bi}_{h}")
            nc.gpsimd.partition_broadcast(bc, recips[h])
            bcs.append(bc)
        for h in range(HPC):
            nc.vector.tensor_mul(
                out=ocat[h * DH:(h + 1) * DH, qb * QB:(qb + 1) * QB],
                in0=av[h][0:DH, :], in1=bcs[h])

    # ---------------- the schedule ----------------
    def F(fn, *a):
        return lambda: fn(*a)

    # Fillers keyed by global beat j; they run after scores[j+2], before AV.
    fillers = {
        0: [F(vproj, 0, 0)],
        1: [F(proj, 0, 1, 1)],
        2: [F(vproj, 0, 1)],
        4: [F(proj, 0, 2, 1)],
        6: [F(vproj, 0, 2)],
        8: [F(proj, 0, 3, 1)],
        10: [F(vproj, 0, 3)],
        12: [F(proj, 0, 1, 0)],
        13: [F(dma_x, 1, 0)],
        16: [F(proj, 0, 2, 0)],
        18: [F(oproj, 0, 0), F(oproj, 0, 1), F(dma_x, 1, 1)],
        19: [F(oproj, 0, 2)],
        20: [F(oproj, 0, 3)],
        21: [F(batch_state, 1)],
        22: [F(proj, 1, 0, 0)],
        24: [F(proj, 1, 0, 1)],
        26: [F(vproj, 1, 0)],
        28: [F(dma_x, 1, 2)],
        32: [F(proj, 0, 3, 0)],
        34: [F(oproj, 0, 4), F(oproj, 0, 5)],
        35: [F(oproj, 0, 6)],
        36: [F(oproj, 0, 7)],
        37: [F(proj, 1, 1, 1)],
        39: [F(vproj, 1, 1)],
        41: [F(dma_x, 1, 3)],
        48: [F(proj, 1, 2, 1)],
        50: [F(oproj, 0, 8), F(oproj, 0, 9)],
        51: [F(vproj, 1, 2)],
        53: [F(oproj, 0, 10)],
        54: [F(oproj, 0, 11)],
        55: [F(proj, 1, 3, 1)],
        57: [F(vproj, 1, 3)],
        74: [F(proj, 1, 1, 0)],
        90: [F(proj, 1, 2, 0)],
        66: [F(oproj, 0, 12), F(oproj, 0, 13)],
        114: [F(oproj, 0, 14)],
        115: [F(oproj, 0, 15)],
        106: [F(proj, 1, 3, 0)],
        82: [F(oproj, 1, 0), F(oproj, 1, 1)],
        83: [F(oproj, 1, 2)],
        84: [F(oproj, 1, 3)],
        98: [F(oproj, 1, 4), F(oproj, 1, 5)],
        99: [F(oproj, 1, 6)],
        100: [F(oproj, 1, 7)],
    }

    # Prologue: first x chunks + fused q/k projection for block 0.
    batch_state(0)
    x0 = xp.tile([P, KD, 512], MM_DT, tag="x", name="x_0_0")
    nc.sync.dma_start(x0[:, 0:2, :], xT_p[:, 0:2, 0:512])
    nc.sync.dma_start(x0[:, 2:5, :], xT_p[:, 2:5, 0:512])
    nc.sync.dma_start(w_sb[:, 4:KD, :], wqkv_p[:, 4:KD, :])
    nc.sync.dma_start(x0[:, 5:KD, :], xT_p[:, 5:KD, 0:512])
    xc[0, 0] = x0
    nc.sync.dma_start(bqk_sb, bqk)
    dma_x(0, 1)
    proj_qk(0, 0)
    dma_x(0, 2)
    dma_x(0, 3)
    nc.sync.dma_start(wo_sb, wo)

    # AV runs one beat behind scores-emission and skips the beat after each
    # block seam, so the previous block's norm (single-buffered av PSUM) gets
    # two full beats to drain; the skipped AV pairs up on the next beat.
    emit_exp(0, emit_scores(0))
    emit_exp(1, emit_scores(1))
    for b in range(NJ):
        if b + 2 < NJ:
            emit_exp(b + 2, emit_scores(b + 2))
        if b % KT == 0 and b > 0:
            # norm first: its DVE ops must not queue behind this beat's
            # filler consumers (DVE is in-order).
            norm(b // KT - 1)
        for f in fillers.get(b, ()):
            f()
        r = b % KT
        if r == KT - 1:
            emit_av(b - 1)           # catch up: norm can start next beat
            emit_av(b)
        elif r == 0:
            pass                     # free beat for the norm to drain
        else:
            emit_av(b - 1)

    # Epilogue: block 6's O tiles run BEFORE block 7's norm is emitted (their
    # ocat reads must not queue behind norm's write — tile-granular deps) and
    # keep the PE busy while the norm drains. Their copies go to ACT so the
    # norm's DVE ops aren't stuck behind them; the final four alternate.
    oproj(1, 8, dual=True)
    oproj(1, 9, dual=True, sc_psum=True)
    oproj(1, 10, dual=True)
    oproj(1, 11, dual=True, sc_psum=True)
    norm(NBLK - 1)
    oproj(1, 12, dual=True)
    oproj(1, 13, dual=True, sc_psum=True)
    oproj(1, 14, dual=True)
    oproj(1, 15, dual=True, sc_psum=True)

    for cm in reversed(ctxs):
        cm.__exit__(None, None, None)


def _bf16_np():
    import ml_dtypes
    return ml_dtypes.bfloat16


def host_inputs(x, W_qkv, b_qkv, W_o, b_o):
    """Per-core input dicts (bf16 activations/weights, fp32 biases)."""
    bf16 = _bf16_np()
    x = np.asarray(x, dtype=np.float32)
    W_qkv = np.asarray(W_qkv, dtype=np.float32)
    b_qkv = np.asarray(b_qkv, dtype=np.float32)
    W_o = np.asarray(W_o, dtype=np.float32)

    xT = np.ascontiguousarray(x.reshape(B * T, D).T).astype(bf16)
    scale = DH ** -0.5
    in_maps = []
    for c in range(NCORES):
        heads = [HPC * c + i for i in range(HPC)]
        cols = []
        biases_qk = []
        for blk, sc in ((0, scale), (1, 1.0)):  # q, k
            for h in heads:
                r = blk * D + h * DH
                cols.append(W_qkv[r:r + DH].T * sc)
                biases_qk.append(b_qkv[r:r + DH] * sc)
        for h in heads:                          # v
            r = 2 * D + h * DH
            cols.append(W_qkv[r:r + DH].T)
        wqkvT = np.ascontiguousarray(np.concatenate(cols, axis=1)).astype(bf16)
        bqk = np.ascontiguousarray(
            np.stack([np.concatenate(biases_qk[:HPC]),
                      np.concatenate(biases_qk[HPC:])], axis=1))
        wo = np.ascontiguousarray(
            np.concatenate([W_o[:, h * DH:(h + 1) * DH] for h in heads],
                           axis=1).T).astype(bf16)
        in_maps.append({"xT": xT, "wqkvT": wqkvT, "bqk": bqk, "wo": wo})
    return in_maps


_NC_CACHE = {}


def get_nc():
    if "nc" not in _NC_CACHE:
        _NC_CACHE["nc"] = build_program()
    return _NC_CACHE["nc"]


def kernel(x, W_qkv, b_qkv, W_o, b_o, _results=None):
    in_maps = host_inputs(x, W_qkv, b_qkv, W_o, b_o)
    if _results is None:
        res = bass_utils.run_bass_kernel_spmd(
            get_nc(), in_maps, core_ids=list(range(NCORES)))
        _results = res.results
    acc = _results[0]["out"].astype(np.float32)
    for c in range(1, NCORES):
        acc = acc + _results[c]["out"].astype(np.float32)
    W_o = np.asarray(W_o, np.float32)
    b_qkv = np.asarray(b_qkv, np.float32)
    bias = np.asarray(b_o, np.float32) + W_o @ b_qkv[2 * D:3 * D]
    acc = acc + bias
    return acc.reshape(B, T, D)



# revision 13
# speedup vs baseline: 1.0150x; 1.0150x over previous
"""Multi-head attention (B=2, T=2048, D=1024, H=16) on 8 TRN2 NeuronCores.

Sharding: tensor-parallel over heads — core c owns heads (2c, 2c+1).
Each core computes its heads' QKV projection (column-sharded), full attention
for those heads, and a row-sharded O-projection partial; the host sums the 8
bf16 partials in fp32 and adds b_o (with W_o @ b_v folded in, since softmax
rows sum to 1).

Host-side prep (bf16 activations/weights, fp32 biases):
  - x is shipped as xT [D, B*T] so D (the contraction dim) lands on
    partitions.
  - W_qkv head-slices are shipped as lhsT [D, 384] with the softmax scale
    folded into the q columns; W_o slice shipped as rhs [128, D].

On-device layout (per batch):
  qkv_T [128, 2, 2048]: q rows (h0 dims 0-63, h1 dims 64-127) and k rows.
  v is projected directly transposed (per 128-token tile the x slice is the
  stationary operand) into v_sb [128 keys, kt, 2*65] with a ones column per
  head, so the AV matmul's row 64 accumulates the softmax denominators.
  Scores are computed transposed [keys, queries] so softmax exp needs no
  transposes; both heads' scores share one [128, 2, 512] PSUM tile so a
  single 1024-wide exp serves a step; no max subtraction (scores ~ N(0,
  0.33) for this init); normalization broadcasts 1/sum across partitions via
  gpsimd, phase-interleaved across heads for the in-order DVE queue.

Schedule: one flat software pipeline over the 128 (batch, query-block,
key-tile) attention steps (query blocks of 512). Per beat the PE runs
scores[j+2], filler work (QKV/V-projection chunks and O-projection tiles —
in their own PSUM tag, so they never recycle a scores bank), then AV one
beat behind; the scalar engine's exp[j] therefore always has two beats of
slack and the scores PSUM a two-beat reuse distance. AV emission skips the
first beat of each block seam (catching up with a paired emission at the
block end) so the previous block's normalization (single-buffered av PSUM)
can drain. PSUM budget:
scores 2x2 banks, av 2x1, projections 2x1. The epilogue's O tiles copy out
on DVE and ACT in parallel and borrow the drained scores PSUM.
"""

import numpy as np

import concourse.bacc as bacc
import concourse.mybir as mybir
import concourse.tile as tile
from concourse import bass_utils

F32 = mybir.dt.float32
BF16 = mybir.dt.bfloat16

B, T, D, H, DH = 2, 2048, 1024, 16, 64
P = 128
NCORES = 8
HPC = H // NCORES          # heads per core = 2
KT = T // P                # key tiles per batch = 16
QB = 512                   # query block
NQB = T // QB              # query blocks per batch = 4
KD = D // P                # contraction tiles for projections = 8
NBLK = B * NQB             # attention blocks = 8
NJ = NBLK * KT             # global spine steps = 128

MM_DT = BF16               # projection matmul dtype
PV_DT = BF16               # probs + v + q/k dtype


def build_program():
    nc = bacc.Bacc(
        "TRN2",
        target_bir_lowering=False,
        debug=False,
        enable_asserts=False,
        num_devices=NCORES,
    )
    xT = nc.dram_tensor("xT", [D, B * T], MM_DT, kind="ExternalInput").ap()
    wqkvT = nc.dram_tensor("wqkvT", [D, 3 * P], MM_DT, kind="ExternalInput").ap()
    bqk = nc.dram_tensor("bqk", [P, 2], F32, kind="ExternalInput").ap()
    wo = nc.dram_tensor("wo", [P, D], MM_DT, kind="ExternalInput").ap()
    out = nc.dram_tensor("out", [B * T, D], BF16, kind="ExternalOutput").ap()

    with tile.TileContext(nc) as tc:
        _body(tc, xT, wqkvT, bqk, wo, out)
    nc.compile()
    return nc


def _body(tc, xT, wqkvT, bqk, wo, out):
    nc = tc.nc
    ctxs = []

    def pool(name, bufs, space="SBUF"):
        cm = tc.tile_pool(name=name, bufs=bufs, space=space)
        p = cm.__enter__()
        ctxs.append(cm)
        return p

    const = pool("const", 1)
    xp = pool("xp", 6)             # x [128,8,512] bf16 chunk tiles (6 live)
    qkvp = pool("qkvp", 2)
    vp = pool("vp", 2)
    probsp = pool("probsp", 8)     # per-(step, head) probs tiles
    ocatp = pool("ocatp", 2)
    outp = pool("outp", 6)
    recipp = pool("recipp", 4)
    bcp = pool("bcp", 4)
    ps = pool("ps", 1, space="PSUM")   # sc: 4 banks, av: 2, pj: 2

    def ps_sc(name):
        # Both heads' scores for one step side by side: one exp instruction
        # covers 1024 elements, halving ACT instruction count and gates.
        return ps.tile([P, HPC, QB], F32, tag="sc", name=name, bufs=2)

    def ps_pj(name):
        return ps.tile([P, QB], F32, tag="pj", name=name, bufs=2)

    # ---- constants ----
    w_sb = const.tile([P, KD, 3 * P], MM_DT, name="w_sb")
    wqkv_p = wqkvT.rearrange("(ko p) m -> p ko m", p=P)
    bqk_sb = const.tile([P, 2], F32, name="bqk_sb")
    wo_sb = const.tile([P, D], MM_DT, name="wo_sb")

    xT_p = xT.rearrange("(ko p) t -> p ko t", p=P)

    # ---------------- per-batch state + work units ----------------
    st = {}
    xc = {}

    def batch_state(b):
        qkvT = qkvp.tile([P, 3, T], PV_DT, tag="qkv", name=f"qkv_{b}")
        v_sb = vp.tile([P, KT, 2 * (DH + 1)], PV_DT, tag="v", name=f"v_{b}")
        v4 = v_sb.rearrange("p t (g c) -> p t g c", g=2)
        nc.vector.memset(v4[:, :, :, DH:DH + 1], 1.0)
        ocat = ocatp.tile([P, T], MM_DT, tag="ocat", name=f"ocat_{b}")
        st[b] = dict(qkvT=qkvT, v4=v4, ocat=ocat)

    def dma_x(b, n):
        """Fetch 512-token chunk n of batch b as two 4-k-tile DMAs."""
        x_t = xp.tile([P, KD, 512], MM_DT, tag="x", name=f"x_{b}_{n}")
        cols = slice(b * T + n * 512, b * T + (n + 1) * 512)
        nc.sync.dma_start(x_t[:, 0:4, :], xT_p[:, 0:4, cols])
        nc.sync.dma_start(x_t[:, 4:KD, :], xT_p[:, 4:KD, cols])
        xc[b, n] = x_t

    def _proj_consume(b, n, m, pq):
        dst = st[b]["qkvT"][:, m, n * 512:(n + 1) * 512]
        if m < 2:
            nc.vector.tensor_scalar_add(dst, pq, bqk_sb[:, m:m + 1])
        else:
            nc.vector.tensor_copy(out=dst, in_=pq)

    def proj(b, n, m):
        """P unit: project chunk n into qkvT[:, m] (8 accum MMs + consumer)."""
        pq = ps_pj(f"qkvps_{b}_{m}_{n}")
        for k in range(KD):
            nc.tensor.matmul(
                pq,
                w_sb[:, k, m * P:(m + 1) * P],
                xc[b, n][:, k, :],
                start=(k == 0),
                stop=(k == KD - 1),
            )
        _proj_consume(b, n, m, pq)

    def proj_qk(b, n):
        """Fused q+k projection of chunk n: both consume x k-tiles as they
        land, so the prologue is paced by one DMA stream, not two passes."""
        pq = [ps_pj(f"qkvps_{b}_0_{n}"),
              ps_sc(f"qkvps_{b}_1_{n}")[:, 0, :]]
        for k in range(KD):
            for m in range(2):
                nc.tensor.matmul(
                    pq[m],
                    w_sb[:, k, m * P:(m + 1) * P],
                    xc[b, n][:, k, :],
                    start=(k == 0),
                    stop=(k == KD - 1),
                )
        for m in range(2):
            _proj_consume(b, n, m, pq[m])

    def vproj(b, n):
        """V unit: project chunk n directly transposed — per 128-token tile,
        the x slice is the stationary operand, so the PSUM comes out
        [tokens, vdims] and no PE transpose is needed."""
        v4 = st[b]["v4"]
        pv = ps_pj(f"vp_{b}_{n}")
        pvt = pv.rearrange("p (t c) -> p t c", t=4)
        for tt in range(4):
            for k in range(KD):
                nc.tensor.matmul(
                    pvt[:, tt, :],
                    xc[b, n][:, k, tt * P:(tt + 1) * P],
                    w_sb[:, k, 2 * P:3 * P],
                    start=(k == 0),
                    stop=(k == KD - 1),
                )
        nc.vector.tensor_copy(
            out=v4[:, 4 * n:4 * n + 4, :, 0:DH],
            in_=pv.bitcast(F32).rearrange("p (t g c) -> p t g c", t=4, g=2),
        )

    def oproj(b, tt, on_act=False, sc_psum=False, dual=False):
        """O unit: project ocat token-tile tt, copy out halves, one DMA.

        sc_psum borrows a scores-tag PSUM tile (both halves side by side);
        dual puts one half's copy on DVE and the other on ACT — both only
        safe in the epilogue once the scores/exp streams have drained.
        """
        ocat = st[b]["ocat"]
        po2 = ps_sc(f"op2_{b}_{tt}") if sc_psum else None
        ob = outp.tile([P, 2, 512], BF16, tag="ob", name=f"ob_{b}_{tt}")
        for nn in range(D // 512):
            po = po2[:, nn, :] if sc_psum else ps_pj(f"op_{b}_{tt}_{nn}")
            nc.tensor.matmul(
                po,
                ocat[:, tt * P:(tt + 1) * P],
                wo_sb[:, nn * 512:(nn + 1) * 512],
                start=True,
                stop=True,
            )
            if on_act or (dual and nn == 1):
                nc.scalar.activation(ob[:, nn, :], po,
                                     mybir.ActivationFunctionType.Copy)
            else:
                nc.vector.tensor_copy(out=ob[:, nn, :], in_=po)
        nc.sync.dma_start(
            out[b * T + tt * P: b * T + (tt + 1) * P, :],
            ob.rearrange("p a b -> p (a b)"))

    # ---------------- attention spine (global steps j = 0..127) -------------
    # step j -> block bi = j // KT -> (b, qb) = divmod(bi, NQB), kt = j % KT
    blocks = [None] * NBLK
    probs = {}   # j -> [tile_h0, tile_h1]

    def block_begin(bi):
        blocks[bi] = [
            ps.tile([DH + 1, QB], F32, tag=f"av{h}",
                    name=f"av_{bi}_{h}", bufs=1) for h in range(HPC)]

    def emit_scores(j):
        bi, kt = divmod(j, KT)
        if blocks[bi] is None:
            block_begin(bi)
        b, qb = divmod(bi, NQB)
        qkvT = st[b]["qkvT"]
        q0 = qb * QB
        s = ps_sc(f"s_{bi}_{kt}")
        for h in range(HPC):
            hs = h * DH
            nc.tensor.matmul(
                s[:, h, :],
                qkvT[hs:hs + DH, 1, kt * P:(kt + 1) * P],
                qkvT[hs:hs + DH, 0, q0:q0 + QB],
                start=True,
                stop=True,
                tile_position=(hs, 0),
            )
        return s

    def emit_exp(j, ps_cur):
        pt = probsp.tile([P, HPC, QB], PV_DT, tag="probs",
                         name=f"pb_{j}", bufs=5)
        nc.scalar.activation(pt, ps_cur,
                             mybir.ActivationFunctionType.Exp)
        probs[j] = pt

    def emit_av(j):
        bi, kt = divmod(j, KT)
        b = bi // NQB
        v4 = st[b]["v4"]
        av = blocks[bi]
        for h in range(HPC):
            nc.tensor.matmul(
                av[h],
                v4[:, kt, h, :],  # [128, 65]
                probs[j][:, h, :],
                start=(kt == 0),
                stop=(kt == KT - 1),
            )
        del probs[j]

    def norm(bi):
        """Normalize both heads of block bi into ocat. Phases interleave so
        the second head's reciprocal is not stuck behind the first head's
        multiply in the in-order DVE queue."""
        b, qb = divmod(bi, NQB)
        ocat = st[b]["ocat"]
        av = blocks[bi]
        recips, bcs = [], []
        for h in range(HPC):
            r = recipp.tile([1, QB], F32, tag="recip", name=f"rc_{bi}_{h}")
            nc.vector.reciprocal(r, av[h][DH:DH + 1, :])
            recips.append(r)
        for h in range(HPC):
            bc = bcp.tile([DH, QB], F32, tag="bc", name=f"bc_{bi}_{h}")
            nc.gpsimd.partition_broadcast(bc, recips[h])
            bcs.append(bc)
        for h in range(HPC):
            nc.vector.tensor_mul(
                out=ocat[h * DH:(h + 1) * DH, qb * QB:(qb + 1) * QB],
                in0=av[h][0:DH, :], in1=bcs[h])

    # ---------------- the schedule ----------------
    def F(fn, *a):
        return lambda: fn(*a)

    # Fillers keyed by global beat j; they run after scores[j+2], before AV.
    fillers = {
        0: [F(vproj, 0, 0)],
        1: [F(proj, 0, 1, 1)],
        2: [F(vproj, 0, 1)],
        4: [F(proj, 0, 2, 1)],
        6: [F(vproj, 0, 2)],
        8: [F(proj, 0, 3, 1)],
        10: [F(vproj, 0, 3)],
        12: [F(proj, 0, 1, 0)],
        13: [F(dma_x, 1, 0)],
        16: [F(proj, 0, 2, 0)],
        18: [F(oproj, 0, 0), F(oproj, 0, 1), F(dma_x, 1, 1)],
        19: [F(oproj, 0, 2)],
        20: [F(oproj, 0, 3)],
        21: [F(batch_state, 1)],
        22: [F(proj, 1, 0, 0)],
        24: [F(proj, 1, 0, 1)],
        26: [F(vproj, 1, 0)],
        28: [F(dma_x, 1, 2)],
        32: [F(proj, 0, 3, 0)],
        34: [F(oproj, 0, 4), F(oproj, 0, 5)],
        35: [F(oproj, 0, 6)],
        36: [F(oproj, 0, 7)],
        37: [F(proj, 1, 1, 1)],
        39: [F(vproj, 1, 1)],
        41: [F(dma_x, 1, 3)],
        48: [F(proj, 1, 2, 1)],
        50: [F(oproj, 0, 8), F(oproj, 0, 9)],
        51: [F(vproj, 1, 2)],
        53: [F(oproj, 0, 10)],
        54: [F(oproj, 0, 11)],
        55: [F(proj, 1, 3, 1)],
        57: [F(vproj, 1, 3)],
        64: [F(proj, 1, 1, 0)],
        80: [F(proj, 1, 2, 0)],
        66: [F(oproj, 0, 12), F(oproj, 0, 13)],
        112: [F(oproj, 0, 14)],
        113: [F(oproj, 0, 15)],
        114: [F(oproj, 1, 8)],
        115: [F(oproj, 1, 9)],
        96: [F(proj, 1, 3, 0)],
        82: [F(oproj, 1, 0), F(oproj, 1, 1)],
        83: [F(oproj, 1, 2)],
        84: [F(oproj, 1, 3)],
        98: [F(oproj, 1, 4), F(oproj, 1, 5)],
        99: [F(oproj, 1, 6)],
        100: [F(oproj, 1, 7)],
    }

    # Prologue: first x chunks + fused q/k projection for block 0. The first
    # w/x k-tiles ship as small interleaved DMAs so the first matmul starts
    # ~2us sooner; later tiles arrive faster than the PE consumes them.
    batch_state(0)
    x0 = xp.tile([P, KD, 512], MM_DT, tag="x", name="x_0_0")
    nc.sync.dma_start(x0[:, 0:1, :], xT_p[:, 0:1, 0:512])
    nc.sync.dma_start(w_sb[:, 0:2, :], wqkv_p[:, 0:2, :])
    nc.sync.dma_start(x0[:, 1:3, :], xT_p[:, 1:3, 0:512])
    nc.sync.dma_start(w_sb[:, 2:4, :], wqkv_p[:, 2:4, :])
    nc.sync.dma_start(x0[:, 3:5, :], xT_p[:, 3:5, 0:512])
    nc.sync.dma_start(w_sb[:, 4:KD, :], wqkv_p[:, 4:KD, :])
    nc.sync.dma_start(x0[:, 5:KD, :], xT_p[:, 5:KD, 0:512])
    xc[0, 0] = x0
    nc.sync.dma_start(bqk_sb, bqk)
    dma_x(0, 1)
    proj_qk(0, 0)
    dma_x(0, 2)
    dma_x(0, 3)
    nc.sync.dma_start(wo_sb, wo)

    # AV runs one beat behind scores-emission and skips the beat after each
    # block seam, so the previous block's norm (single-buffered av PSUM) gets
    # two full beats to drain; the skipped AV pairs up on the next beat.
    emit_exp(0, emit_scores(0))
    emit_exp(1, emit_scores(1))
    for b in range(NJ):
        if b + 2 < NJ:
            emit_exp(b + 2, emit_scores(b + 2))
        if b % KT == 0 and b > 0:
            # norm first: its DVE ops must not queue behind this beat's
            # filler consumers (DVE is in-order).
            norm(b // KT - 1)
        for f in fillers.get(b, ()):
            f()
        r = b % KT
        if r == KT - 1:
            emit_av(b - 1)           # catch up: norm can start next beat
            emit_av(b)
        elif r == 0:
            pass                     # free beat for the norm to drain
        else:
            emit_av(b - 1)

    # Epilogue: blocks 5/6's remaining O tiles run BEFORE block 7's norm is
    # emitted (their ocat reads must not queue behind norm's write —
    # tile-granular deps) and keep the PE busy while the norm drains. The
    # first two put both copy halves on ACT so the norm's DVE ops (emitted
    # right after) start immediately; the rest alternate DVE/ACT.
    oproj(1, 10, on_act=True)
    oproj(1, 11, on_act=True, sc_psum=True)
    norm(NBLK - 1)
    # Final four tiles: each gets its own PSUM (pj / the two sc slots / the
    # av banks the norm just drained) so the 8 matmuls run back-to-back with
    # no copy-recycle waits; copies split DVE/ACT; per-tile DMAs pipeline.
    ocat = st[1]["ocat"]
    fin_ps = {}
    fin_ps[12] = [ps_pj("opF_12_0"), ps_pj("opF_12_1")]
    s13 = ps_sc("opF_13")
    fin_ps[13] = [s13[:, 0, :], s13[:, 1, :]]
    s14 = ps_sc("opF_14")
    fin_ps[14] = [s14[:, 0, :], s14[:, 1, :]]
    fin_ps[15] = [
        ps.tile([P, 512], F32, tag="av0", name="opF_15_0", bufs=1),
        ps.tile([P, 512], F32, tag="av1", name="opF_15_1", bufs=1)]
    for tt in (12, 13, 14, 15):
        ob = outp.tile([P, 2, 512], BF16, tag="ob", name=f"obF_{tt}")
        for nn in range(2):
            nc.tensor.matmul(
                fin_ps[tt][nn],
                ocat[:, tt * P:(tt + 1) * P],
                wo_sb[:, nn * 512:(nn + 1) * 512],
                start=True,
                stop=True,
            )
            if nn == 1:
                nc.scalar.activation(ob[:, nn, :], fin_ps[tt][nn],
                                     mybir.ActivationFunctionType.Copy)
            else:
                nc.vector.tensor_copy(out=ob[:, nn, :], in_=fin_ps[tt][nn])
        nc.sync.dma_start(
            out[T + tt * P:T + (tt + 1) * P, :],
            ob.rearrange("p a b -> p (a b)"))

    for cm in reversed(ctxs):
        cm.__exit__(None, None, None)


def _bf16_np():
    import ml_dtypes
    return ml_dtypes.bfloat16


def host_inputs(x, W_qkv, b_qkv, W_o, b_o):
    """Per-core input dicts (bf16 activations/weights, fp32 biases)."""
    bf16 = _bf16_np()
    x = np.asarray(x, dtype=np.float32)
    W_qkv = np.asarray(W_qkv, dtype=np.float32)
    b_qkv = np.asarray(b_qkv, dtype=np.float32)
    W_o = np.asarray(W_o, dtype=np.float32)

    xT = np.ascontiguousarray(x.reshape(B * T, D).T).astype(bf16)
    scale = DH ** -0.5
    in_maps = []
    for c in range(NCORES):
        heads = [HPC * c + i for i in range(HPC)]
        cols = []
        biases_qk = []
        for blk, sc in ((0, scale), (1, 1.0)):  # q, k
            for h in heads:
                r = blk * D + h * DH
                cols.append(W_qkv[r:r + DH].T * sc)
                biases_qk.append(b_qkv[r:r + DH] * sc)
        for h in heads:                          # v
            r = 2 * D + h * DH
            cols.append(W_qkv[r:r + DH].T)
        wqkvT = np.ascontiguousarray(np.concatenate(cols, axis=1)).astype(bf16)
        bqk = np.ascontiguousarray(
            np.stack([np.concatenate(biases_qk[:HPC]),
                      np.concatenate(biases_qk[HPC:])], axis=1))
        wo = np.ascontiguousarray(
            np.concatenate([W_o[:, h * DH:(h + 1) * DH] for h in heads],
                           axis=1).T).astype(bf16)
        in_maps.append({"xT": xT, "wqkvT": wqkvT, "bqk": bqk, "wo": wo})
    return in_maps


_NC_CACHE = {}


def get_nc():
    if "nc" not in _NC_CACHE:
        _NC_CACHE["nc"] = build_program()
    return _NC_CACHE["nc"]


def kernel(x, W_qkv, b_qkv, W_o, b_o, _results=None):
    in_maps = host_inputs(x, W_qkv, b_qkv, W_o, b_o)
    if _results is None:
        res = bass_utils.run_bass_kernel_spmd(
            get_nc(), in_maps, core_ids=list(range(NCORES)))
        _results = res.results
    acc = _results[0]["out"].astype(np.float32)
    for c in range(1, NCORES):
        acc = acc + _results[c]["out"].astype(np.float32)
    W_o = np.asarray(W_o, np.float32)
    b_qkv = np.asarray(b_qkv, np.float32)
    bias = np.asarray(b_o, np.float32) + W_o @ b_qkv[2 * D:3 * D]
    acc = acc + bias
    return acc.reshape(B, T, D)

